# revision 26
# baseline (speedup 1.0000x reference)
"""Trainium2 Bass kernel for DistanceSelfAttention.

Computation (per batch b):
    q/k/v = x @ w{q,k,v}.T + b{q,k,v}            -> [N, E], heads H=8, D=64
    sc    = clip(q k^T / sqrt(D) + db, -10, 10)
    sc    = where(mask[j], sc, -1e9)             (key-side mask)
    a     = softmax(sc, axis=-1)
    out   = (a v) @ wo.T + bo

Sharding: pure data-parallel over batch B=16 across 8 cores (2 per core),
weights replicated, no collectives.

Device-side design (per local batch):
    xT  [e, i]   - x transposed (host-prepped)
    QT/KT [e',i] - projections with output-dim on partitions; bias (+1/sqrt(D)
                   scale for Q) fused into the PSUM->SBUF evacuation
    V   [j, e_v] - token-major, head-split with a trailing ones column, rows
                   scaled by the 0/1 key mask: the AV matmul then yields both
                   the masked numerator and the masked softmax denominator
                   (last PSUM row) in one pass
    S.T [j, i]   - scores transposed; db.T injected into each head's PSUM via
                   an identity matmul, then the K=64 QK product accumulates
                   on top.  exp runs on ACT directly from PSUM; clip is
                   applied *after* exp as clamp(e, e^-10, e^+10) on GpSimd
                   (monotonicity makes them equivalent; masking lives in V)
    O.T [e, i]   - AV output, normalized by reciprocal denominators broadcast
                   across partitions
All matmuls run as float32r (TF32-like, 1 PE cycle/row vs 4 for fp32).
Emission is software-pipelined: head-pair p+1's scores are issued before
pair p's AV, and batch 1's projections are issued inside batch 0's
attention tail so the PE never drains.
"""

import sys

sys.path.insert(0, "/opt/trn_rl_repo")

import numpy as np

import concourse.bass as bass  # noqa: F401
import concourse.tile as tile
from concourse import bacc, mybir
from concourse.bass_utils import run_bass_kernel_spmd

B, N, E, H = 16, 512, 512, 8
D = E // H
P = 128
NCORES = 8
BPC = B // NCORES  # batches per core
NT = N // P        # token tiles
ET = E // P        # embedding tiles
HP = H // 2        # head pairs
F32 = mybir.dt.float32
F32R = mybir.dt.float32r
BF16 = mybir.dt.bfloat16
AX = mybir.AluOpType
AF = mybir.ActivationFunctionType
EXP10 = float(np.exp(10.0))
EXPM10 = float(np.exp(-10.0))


def build_nc(debug_taps=False):
    nc = bacc.Bacc("TRN2", target_bir_lowering=False, debug=False,
                   num_devices=NCORES)

    xT = nc.dram_tensor("xT", [BPC, E, N], F32R, kind="ExternalInput")
    dbT = nc.dram_tensor("dbT", [BPC, N, N], F32R, kind="ExternalInput")
    m01 = nc.dram_tensor("m01", [BPC, N], F32, kind="ExternalInput")
    wqT = nc.dram_tensor("wqT", [E, E], F32R, kind="ExternalInput")
    wkT = nc.dram_tensor("wkT", [E, E], F32R, kind="ExternalInput")
    wvT = nc.dram_tensor("wvT", [E, E], F32R, kind="ExternalInput")
    woT = nc.dram_tensor("woT", [E, E], F32R, kind="ExternalInput")
    bq = nc.dram_tensor("bq", [E], F32, kind="ExternalInput")
    bk = nc.dram_tensor("bk", [E], F32, kind="ExternalInput")
    bv = nc.dram_tensor("bv", [E], F32, kind="ExternalInput")
    bo = nc.dram_tensor("bo", [E], F32, kind="ExternalInput")
    ident = nc.dram_tensor("ident", [P, P], F32R, kind="ExternalInput")
    out = nc.dram_tensor("out", [BPC, N, E], F32, kind="ExternalOutput")

    with tile.TileContext(nc) as tc:
        with (
            tc.tile_pool(name="wpool", bufs=1) as wpool,
            tc.tile_pool(name="cpool", bufs=1) as cpool,
            tc.tile_pool(name="xpool", bufs=2) as xpool,
            tc.tile_pool(name="dbpool", bufs=2) as dbpool,
            tc.tile_pool(name="qkpool", bufs=2) as qkpool,
            tc.tile_pool(name="vpool", bufs=2) as vpool,
            tc.tile_pool(name="epool", bufs=2) as epool,
            tc.tile_pool(name="otpool", bufs=2) as otpool,
            tc.tile_pool(name="nrm", bufs=3) as nrm,
            tc.tile_pool(name="opool", bufs=3) as opool,
            tc.tile_pool(name="scps", bufs=2, space="PSUM") as scps,
            tc.tile_pool(name="avps", bufs=2, space="PSUM") as avps,
            tc.tile_pool(name="mmps", bufs=2, space="PSUM") as mmps,
        ):
            # ---- resident weights / constants ----
            # Weights stream on the ACT HWDGE ring (idle during the head);
            # xT/dbT stream on the SP ring so the first projection matmul
            # is gated only by wq chunk 0 + xT chunk 0.
            w_sb = {}
            w_src = {}
            for name, t in (("wq", wqT), ("wk", wkT), ("wv", wvT),
                            ("wo", woT)):
                w_sb[name] = wpool.tile([P, ET, E], F32R, tag=f"w_{name}",
                                        name=name)
                w_src[name] = t.ap().rearrange("(kt p) o -> p kt o", p=P)

            def load_w(name, engs=(None,)):
                for kt in range(ET):
                    eng = engs[kt % len(engs)]
                    eng.dma_start(w_sb[name][:, kt, :],
                                  w_src[name][:, kt, :])

            load_w("wq", (nc.scalar, nc.sync))
            load_w("wk", (nc.sync, nc.scalar))
            load_w("wv", (nc.scalar, nc.sync))
            load_w("wo", (nc.sync, nc.scalar))
            bq_sb = cpool.tile([P, ET], F32, tag="bq")
            nc.gpsimd.dma_start(bq_sb[:],
                                bq.ap().rearrange("(t p) -> p t", p=P))
            bk_sb = cpool.tile([P, ET], F32, tag="bk")
            nc.gpsimd.dma_start(bk_sb[:],
                                bk.ap().rearrange("(t p) -> p t", p=P))
            bv_sb = cpool.tile([P, E], F32, tag="bv")
            nc.gpsimd.dma_start(bv_sb[:],
                                bv.ap()[None, :].broadcast_to([P, E]))
            bo_sb = cpool.tile([P, E], F32, tag="bo")
            nc.gpsimd.dma_start(bo_sb[:],
                                bo.ap()[None, :].broadcast_to([P, E]))
            id_sb = cpool.tile([P, P], F32R, tag="ident")
            nc.gpsimd.dma_start(id_sb[:], ident.ap())

            dbg = {}
            if debug_taps:
                for nm, shp in (("d_qt", [P, ET, N]), ("d_kt", [P, ET, N]),
                                ("d_v", [P, NT, H, D + 1]),
                                ("d_e", [H, P, NT, N]),
                                ("d_av", [H, D + 1, N]),
                                ("d_ot", [P, ET, N])):
                    dbg[nm] = nc.dram_tensor(nm, shp, F32,
                                             kind="ExternalOutput")

            st = [dict() for _ in range(BPC)]  # per-batch live tiles

            def load(b, first=False):
                xT_sb = xpool.tile([P, ET, N], F32R, tag="xT", name="xT_sb")
                xr = xT.ap()[b].rearrange("(kt p) i -> p kt i", p=P)
                for kt in range(ET):
                    nc.sync.dma_start(xT_sb[:, kt, :], xr[:, kt, :])
                dbT_sb = dbpool.tile([P, NT, N], F32R, tag="dbT",
                                     name="dbT_sb")
                dr = dbT.ap()[b].rearrange("(jt p) i -> p jt i", p=P)
                for jt in range(NT):
                    nc.sync.dma_start(dbT_sb[:, jt, :], dr[:, jt, :])
                m01_sb = cpool.tile([P, NT], F32, tag=f"m01{b}",
                                    name="m01_sb")
                nc.gpsimd.dma_start(m01_sb[:], m01.ap()[b].rearrange(
                    "(t p) -> p t", p=P))
                st[b].update(xT=xT_sb, dbT=dbT_sb, m01=m01_sb)

            def proj(b):
                s = st[b]
                xT_sb = s["xT"]
                qt_sb = qkpool.tile([P, ET, N], F32R, tag="qt", name="qt_sb")
                kt_sb = qkpool.tile([P, ET, N], F32R, tag="kt", name="kt_sb")
                for et in range(ET):
                    for wname, dst, bias, scl in (
                            ("wq", qt_sb, bq_sb, float(1 / np.sqrt(D))),
                            ("wk", kt_sb, bk_sb, None)):
                        ps = mmps.tile([P, N], F32, tag="proj", name="ps")
                        for ke in range(ET):
                            nc.tensor.matmul(
                                ps[:],
                                w_sb[wname][:, ke, et * P:(et + 1) * P],
                                xT_sb[:, ke, :],
                                start=(ke == 0), stop=(ke == ET - 1))
                        if scl is not None:
                            nc.vector.tensor_scalar(
                                dst[:, et, :], ps[:], bias[:, et:et + 1],
                                scl, AX.add, AX.mult)
                        else:
                            nc.vector.tensor_scalar(
                                dst[:, et, :], ps[:], bias[:, et:et + 1],
                                None, AX.add)
                v_sb = vpool.tile([P, NT, H, D + 1], BF16, tag="v",
                                  name="v_sb")
                for jt in range(NT):
                    ps = mmps.tile([P, N], F32, tag="proj", name="ps")
                    for ke in range(ET):
                        nc.tensor.matmul(
                            ps[:],
                            xT_sb[:, ke, jt * P:(jt + 1) * P],
                            w_sb["wv"][:, ke, :],
                            start=(ke == 0), stop=(ke == ET - 1))
                    nc.vector.tensor_add(
                        v_sb[:, jt, :, 0:D],
                        ps[:].rearrange("p (h d) -> p h d", h=H),
                        bv_sb[:].rearrange("p (h d) -> p h d", h=H))
                    nc.vector.memset(v_sb[:, jt, :, D:D + 1], 1.0)
                    # key mask: zero masked token rows (incl. ones column)
                    nc.vector.tensor_scalar(
                        v_sb[:, jt, :, :], v_sb[:, jt, :, :],
                        s["m01"][:, jt:jt + 1], None, AX.mult)
                s.update(qt=qt_sb, kt=kt_sb, v=v_sb)

            def scores(b, hp):
                """Head pair (2hp, 2hp+1): db-injected, exp'd score tiles."""
                s = st[b]
                e_ab = (epool.tile([P, NT, N], BF16, tag="eA", name="eA"),
                        epool.tile([P, NT, N], BF16, tag="eB", name="eB"))
                for jt in range(NT):
                    sc_ab = (scps.tile([P, N], F32, tag="scA", name="scA"),
                             scps.tile([P, N], F32, tag="scB", name="scB"))
                    for sc in sc_ab:
                        nc.tensor.matmul(sc[:], id_sb[:], s["dbT"][:, jt, :],
                                         start=True, stop=False)
                    for half, sc in enumerate(sc_ab):
                        of = half * D
                        nc.tensor.matmul(
                            sc[:],
                            s["kt"][of:of + D, hp, jt * P:(jt + 1) * P],
                            s["qt"][of:of + D, hp, :],
                            start=False, stop=True, tile_position=(of, 0))
                    for half, sc in enumerate(sc_ab):
                        nc.scalar.activation(e_ab[half][:, jt, :], sc[:],
                                             AF.Exp)
                        nc.vector.tensor_scalar(
                            e_ab[half][:, jt, :], e_ab[half][:, jt, :],
                            EXP10, EXPM10, AX.min, AX.max)
                return e_ab

            def av_norm(b, hp, e_ab):
                s = st[b]
                for half, e_sb in enumerate(e_ab):
                    h = 2 * hp + half
                    av = avps.tile([D + 1, N], F32, tag="av", name="av")
                    for jt in range(NT):
                        nc.tensor.matmul(av[:], s["v"][:, jt, h, :],
                                         e_sb[:, jt, :],
                                         start=(jt == 0), stop=(jt == NT - 1))
                    if debug_taps and b == 0:
                        nc.sync.dma_start(dbg["d_e"].ap()[h],
                                          e_sb[:].bitcast(F32))
                        av_dbg = nrm.tile([D + 1, N], F32, tag="av_dbg",
                                          name="av_dbg")
                        nc.vector.tensor_copy(av_dbg[:], av[:])
                        nc.sync.dma_start(dbg["d_av"].ap()[h], av_dbg[:])
                    den0 = nrm.tile([1, N], F32, tag="den0", name="den0")
                    nc.scalar.copy(den0[:], av[D:D + 1, :])
                    rcp = nrm.tile([1, N], F32, tag="rcp", name="rcp")
                    nc.vector.reciprocal_approx_fast(rcp[:], den0[:])
                    rbc = nrm.tile([D, N], F32, tag="rbc", name="rbc")
                    nc.gpsimd.partition_broadcast(rbc[:], rcp[:])
                    nc.vector.tensor_mul(
                        s["ot"][(h % 2) * D:(h % 2) * D + D, h // 2, :],
                        av[0:D, :], rbc[:])

            def final(b):
                s = st[b]
                if debug_taps and b == 0:
                    nc.sync.dma_start(dbg["d_qt"].ap(),
                                      s["qt"][:].bitcast(F32))
                    nc.sync.dma_start(dbg["d_kt"].ap(),
                                      s["kt"][:].bitcast(F32))
                    nc.sync.dma_start(dbg["d_v"].ap(), s["v"][:].bitcast(F32))
                    nc.sync.dma_start(dbg["d_ot"].ap(),
                                      s["ot"][:].bitcast(F32))
                for it in range(NT):
                    ps = mmps.tile([P, N], F32, tag="proj", name="ps")
                    for et in range(ET):
                        nc.tensor.matmul(
                            ps[:],
                            s["ot"][:, et, it * P:(it + 1) * P],
                            w_sb["wo"][:, et, :],
                            start=(et == 0), stop=(et == ET - 1))
                    o_sb = opool.tile([P, N], F32, tag="o", name="o_sb")
                    nc.vector.tensor_add(o_sb[:], ps[:], bo_sb[:])
                    nc.sync.dma_start(out.ap()[b, it * P:(it + 1) * P, :],
                                      o_sb[:])

            # ---- emission schedule (PE program order) ----
            load(0, first=True)
            proj(0)
            st[0]["ot"] = otpool.tile([P, ET, N], F32R, tag="ot", name="ot0")
            e_prev = scores(0, 0)
            for hp in range(1, HP):
                e_cur = scores(0, hp)
                av_norm(0, hp - 1, e_prev)
                e_prev = e_cur
            load(1)
            proj(1)  # fills the PE while batch 0's last exp chain drains
            av_norm(0, HP - 1, e_prev)
            st[1]["ot"] = otpool.tile([P, ET, N], F32R, tag="ot", name="ot1")
            e_prev = scores(1, 0)
            final(0)  # after scores(1,0) so the PE rides over b0's norm tail
            for hp in range(1, HP):
                e_cur = scores(1, hp)
                av_norm(1, hp - 1, e_prev)
                e_prev = e_cur
            av_norm(1, HP - 1, e_prev)
            final(1)
    nc.compile()
    return nc


_NC = None


def _get_nc():
    global _NC
    if _NC is None:
        _NC = build_nc()
    return _NC


def _prep_in_maps(x, db, mask, wq, bq, wk, bk, wv, bv, wo, bo):
    f = np.float32
    x = np.asarray(x, f)
    db = np.asarray(db, f)
    mask = np.asarray(mask)
    xTa = np.ascontiguousarray(x.transpose(0, 2, 1))
    dbTa = np.ascontiguousarray(db.transpose(0, 2, 1))
    m01a = (mask != 0).astype(f)
    consts = dict(
        wqT=np.ascontiguousarray(np.asarray(wq, f).T),
        wkT=np.ascontiguousarray(np.asarray(wk, f).T),
        wvT=np.ascontiguousarray(np.asarray(wv, f).T),
        woT=np.ascontiguousarray(np.asarray(wo, f).T),
        bq=np.asarray(bq, f),
        bk=np.asarray(bk, f),
        bv=np.asarray(bv, f),
        bo=np.asarray(bo, f),
        ident=np.eye(P, dtype=f),
    )
    in_maps = []
    for c in range(NCORES):
        s = slice(c * BPC, (c + 1) * BPC)
        in_maps.append(dict(xT=xTa[s], dbT=dbTa[s], m01=m01a[s], **consts))
    return in_maps


def _install_ntff_hook():
    """The agent image's antenv lacks axon_hooks; provide a shim so
    run_bass_kernel_spmd(trace=True) can capture NTFF profiles."""
    import types

    if "antenv.axon_hooks" in sys.modules:
        return
    try:
        from trn_agent_boot.trn_boot import _ntff_profile_via_ctypes
        hook = _ntff_profile_via_ctypes("/opt/axon/libaxon_pjrt.so")
    except Exception:
        hook = None
    mod = types.ModuleType("antenv.axon_hooks")
    mod.get_axon_ntff_profile_hook = lambda: hook
    mod.set_axon_ntff_profile_hook = lambda h: None
    sys.modules["antenv.axon_hooks"] = mod


def run(trace=False, **inputs):
    if trace:
        _install_ntff_hook()
    nc = _get_nc()
    in_maps = _prep_in_maps(**inputs)
    res = run_bass_kernel_spmd(nc, in_maps, core_ids=list(range(NCORES)),
                               trace=trace)
    out = np.concatenate([res.results[c]["out"] for c in range(NCORES)],
                         axis=0)
    return out, res


def kernel(**inputs):
    out, _ = run(trace=False, **inputs)
    return out


# revision 27
# speedup vs baseline: 1.0416x; 1.0416x over previous
"""Trainium2 Bass kernel for DistanceSelfAttention.

Computation (per batch b):
    q/k/v = x @ w{q,k,v}.T + b{q,k,v}            -> [N, E], heads H=8, D=64
    sc    = clip(q k^T / sqrt(D) + db, -10, 10)
    sc    = where(mask[j], sc, -1e9)             (key-side mask)
    a     = softmax(sc, axis=-1)
    out   = (a v) @ wo.T + bo

Sharding: pure data-parallel over batch B=16 across 8 cores (2 per core),
weights replicated, no collectives.

Device-side design (per local batch):
    xT  [e, i]   - x transposed (host-prepped)
    QT/KT [e',i] - projections with output-dim on partitions; bias (+1/sqrt(D)
                   scale for Q) fused into the PSUM->SBUF evacuation
    V   [j, e_v] - token-major, head-split with a trailing ones column, rows
                   scaled by the 0/1 key mask: the AV matmul then yields both
                   the masked numerator and the masked softmax denominator
                   (last PSUM row) in one pass
    S.T [j, i]   - scores transposed; db.T injected into each head's PSUM via
                   an identity matmul, then the K=64 QK product accumulates
                   on top.  exp runs on ACT directly from PSUM; clip is
                   applied *after* exp as clamp(e, e^-10, e^+10) on GpSimd
                   (monotonicity makes them equivalent; masking lives in V)
    O.T [e, i]   - AV output, normalized by reciprocal denominators broadcast
                   across partitions
All matmuls run as float32r (TF32-like, 1 PE cycle/row vs 4 for fp32).
Emission is software-pipelined: head-pair p+1's scores are issued before
pair p's AV, and batch 1's projections are issued inside batch 0's
attention tail so the PE never drains.
"""

import sys

sys.path.insert(0, "/opt/trn_rl_repo")

import numpy as np

import concourse.bass as bass  # noqa: F401
import concourse.tile as tile
from concourse import bacc, mybir
from concourse.bass_utils import run_bass_kernel_spmd

B, N, E, H = 16, 512, 512, 8
D = E // H
P = 128
NCORES = 8
BPC = B // NCORES  # batches per core
NT = N // P        # token tiles
ET = E // P        # embedding tiles
HP = H // 2        # head pairs
F32 = mybir.dt.float32
F32R = mybir.dt.float32r
BF16 = mybir.dt.bfloat16
AX = mybir.AluOpType
AF = mybir.ActivationFunctionType
EXP10 = float(np.exp(10.0))
EXPM10 = float(np.exp(-10.0))


def build_nc(debug_taps=False):
    nc = bacc.Bacc("TRN2", target_bir_lowering=False, debug=False,
                   num_devices=NCORES)

    xT = nc.dram_tensor("xT", [BPC, E, N], F32R, kind="ExternalInput")
    dbT = nc.dram_tensor("dbT", [BPC, N, N], F32R, kind="ExternalInput")
    m01 = nc.dram_tensor("m01", [BPC, N], F32, kind="ExternalInput")
    wqT = nc.dram_tensor("wqT", [E, E], F32R, kind="ExternalInput")
    wkT = nc.dram_tensor("wkT", [E, E], F32R, kind="ExternalInput")
    wvT = nc.dram_tensor("wvT", [E, E], F32R, kind="ExternalInput")
    woT = nc.dram_tensor("woT", [E, E], F32R, kind="ExternalInput")
    bq = nc.dram_tensor("bq", [E], F32, kind="ExternalInput")
    bk = nc.dram_tensor("bk", [E], F32, kind="ExternalInput")
    bv = nc.dram_tensor("bv", [E], F32, kind="ExternalInput")
    bo = nc.dram_tensor("bo", [E], F32, kind="ExternalInput")
    ident = nc.dram_tensor("ident", [P, P], F32R, kind="ExternalInput")
    out = nc.dram_tensor("out", [BPC, N, E], F32, kind="ExternalOutput")

    with tile.TileContext(nc) as tc:
        with (
            tc.tile_pool(name="wpool", bufs=1) as wpool,
            tc.tile_pool(name="cpool", bufs=1) as cpool,
            tc.tile_pool(name="xpool", bufs=2) as xpool,
            tc.tile_pool(name="dbpool", bufs=2) as dbpool,
            tc.tile_pool(name="qkpool", bufs=2) as qkpool,
            tc.tile_pool(name="vpool", bufs=2) as vpool,
            tc.tile_pool(name="epool", bufs=2) as epool,
            tc.tile_pool(name="otpool", bufs=2) as otpool,
            tc.tile_pool(name="nrm", bufs=3) as nrm,
            tc.tile_pool(name="opool", bufs=3) as opool,
            tc.tile_pool(name="scps", bufs=2, space="PSUM") as scps,
            tc.tile_pool(name="avps", bufs=2, space="PSUM") as avps,
            tc.tile_pool(name="mmps", bufs=2, space="PSUM") as mmps,
        ):
            # ---- resident weights / constants ----
            # Weights stream on the ACT HWDGE ring (idle during the head);
            # xT/dbT stream on the SP ring so the first projection matmul
            # is gated only by wq chunk 0 + xT chunk 0.
            w_sb = {}
            w_src = {}
            for name, t in (("wq", wqT), ("wk", wkT), ("wv", wvT),
                            ("wo", woT)):
                w_sb[name] = wpool.tile([P, ET, E], F32R, tag=f"w_{name}",
                                        name=name)
                w_src[name] = t.ap().rearrange("(kt p) o -> p kt o", p=P)

            def load_w(name, engs=(None,)):
                for kt in range(ET):
                    eng = engs[kt % len(engs)]
                    eng.dma_start(w_sb[name][:, kt, :],
                                  w_src[name][:, kt, :])

            for _wn in ("wq", "wk", "wv", "wo"):
                load_w(_wn, (nc.scalar,))
            bq_sb = cpool.tile([P, ET], F32, tag="bq")
            nc.gpsimd.dma_start(bq_sb[:],
                                bq.ap().rearrange("(t p) -> p t", p=P))
            bk_sb = cpool.tile([P, ET], F32, tag="bk")
            nc.gpsimd.dma_start(bk_sb[:],
                                bk.ap().rearrange("(t p) -> p t", p=P))
            bv_sb = cpool.tile([P, E], F32, tag="bv")
            nc.gpsimd.dma_start(bv_sb[:],
                                bv.ap()[None, :].broadcast_to([P, E]))
            bo_sb = cpool.tile([P, E], F32, tag="bo")
            nc.gpsimd.dma_start(bo_sb[:],
                                bo.ap()[None, :].broadcast_to([P, E]))
            id_sb = cpool.tile([P, P], F32R, tag="ident")
            nc.gpsimd.dma_start(id_sb[:], ident.ap())

            dbg = {}
            if debug_taps:
                for nm, shp in (("d_qt", [P, ET, N]), ("d_kt", [P, ET, N]),
                                ("d_v", [P, NT, H, D + 1]),
                                ("d_e", [H, P, NT, N]),
                                ("d_av", [H, D + 1, N]),
                                ("d_ot", [P, ET, N])):
                    dbg[nm] = nc.dram_tensor(nm, shp, F32,
                                             kind="ExternalOutput")

            st = [dict() for _ in range(BPC)]  # per-batch live tiles

            def load(b, first=False):
                xT_sb = xpool.tile([P, ET, N], F32R, tag="xT", name="xT_sb")
                xr = xT.ap()[b].rearrange("(kt p) i -> p kt i", p=P)
                for kt in range(ET):
                    nc.sync.dma_start(xT_sb[:, kt, :], xr[:, kt, :])
                dbT_sb = dbpool.tile([P, NT, N], F32R, tag="dbT",
                                     name="dbT_sb")
                dr = dbT.ap()[b].rearrange("(jt p) i -> p jt i", p=P)
                for jt in range(NT):
                    nc.sync.dma_start(dbT_sb[:, jt, :], dr[:, jt, :])
                m01_sb = cpool.tile([P, NT], F32, tag=f"m01{b}",
                                    name="m01_sb")
                nc.gpsimd.dma_start(m01_sb[:], m01.ap()[b].rearrange(
                    "(t p) -> p t", p=P))
                st[b].update(xT=xT_sb, dbT=dbT_sb, m01=m01_sb)

            def proj(b):
                s = st[b]
                xT_sb = s["xT"]
                qt_sb = qkpool.tile([P, ET, N], F32R, tag="qt", name="qt_sb")
                kt_sb = qkpool.tile([P, ET, N], F32R, tag="kt", name="kt_sb")
                for et in range(ET):
                    for wname, dst, bias, scl in (
                            ("wq", qt_sb, bq_sb, float(1 / np.sqrt(D))),
                            ("wk", kt_sb, bk_sb, None)):
                        ps = mmps.tile([P, N], F32, tag="proj", name="ps")
                        for ke in range(ET):
                            nc.tensor.matmul(
                                ps[:],
                                w_sb[wname][:, ke, et * P:(et + 1) * P],
                                xT_sb[:, ke, :],
                                start=(ke == 0), stop=(ke == ET - 1))
                        if scl is not None:
                            nc.vector.tensor_scalar(
                                dst[:, et, :], ps[:], bias[:, et:et + 1],
                                scl, AX.add, AX.mult)
                        else:
                            nc.vector.tensor_scalar(
                                dst[:, et, :], ps[:], bias[:, et:et + 1],
                                None, AX.add)
                v_sb = vpool.tile([P, NT, H, D + 1], BF16, tag="v",
                                  name="v_sb")
                for jt in range(NT):
                    ps = mmps.tile([P, N], F32, tag="proj", name="ps")
                    for ke in range(ET):
                        nc.tensor.matmul(
                            ps[:],
                            xT_sb[:, ke, jt * P:(jt + 1) * P],
                            w_sb["wv"][:, ke, :],
                            start=(ke == 0), stop=(ke == ET - 1))
                    nc.vector.tensor_add(
                        v_sb[:, jt, :, 0:D],
                        ps[:].rearrange("p (h d) -> p h d", h=H),
                        bv_sb[:].rearrange("p (h d) -> p h d", h=H))
                    nc.vector.memset(v_sb[:, jt, :, D:D + 1], 1.0)
                    # key mask: zero masked token rows (incl. ones column)
                    nc.vector.tensor_scalar(
                        v_sb[:, jt, :, :], v_sb[:, jt, :, :],
                        s["m01"][:, jt:jt + 1], None, AX.mult)
                s.update(qt=qt_sb, kt=kt_sb, v=v_sb)

            def scores(b, hp):
                """Head pair (2hp, 2hp+1): db-injected, exp'd score tiles."""
                s = st[b]
                e_ab = (epool.tile([P, NT, N], BF16, tag="eA", name="eA"),
                        epool.tile([P, NT, N], BF16, tag="eB", name="eB"))
                for jt in range(NT):
                    sc_ab = (scps.tile([P, N], F32, tag="scA", name="scA"),
                             scps.tile([P, N], F32, tag="scB", name="scB"))
                    for sc in sc_ab:
                        nc.tensor.matmul(sc[:], id_sb[:], s["dbT"][:, jt, :],
                                         start=True, stop=False)
                    for half, sc in enumerate(sc_ab):
                        of = half * D
                        nc.tensor.matmul(
                            sc[:],
                            s["kt"][of:of + D, hp, jt * P:(jt + 1) * P],
                            s["qt"][of:of + D, hp, :],
                            start=False, stop=True, tile_position=(of, 0))
                    for half, sc in enumerate(sc_ab):
                        nc.scalar.activation(e_ab[half][:, jt, :], sc[:],
                                             AF.Exp)
                        nc.vector.tensor_scalar(
                            e_ab[half][:, jt, :], e_ab[half][:, jt, :],
                            EXP10, EXPM10, AX.min, AX.max)
                return e_ab

            def av_norm(b, hp, e_ab):
                s = st[b]
                for half, e_sb in enumerate(e_ab):
                    h = 2 * hp + half
                    av = avps.tile([D + 1, N], F32, tag="av", name="av")
                    for jt in range(NT):
                        nc.tensor.matmul(av[:], s["v"][:, jt, h, :],
                                         e_sb[:, jt, :],
                                         start=(jt == 0), stop=(jt == NT - 1))
                    if debug_taps and b == 0:
                        nc.sync.dma_start(dbg["d_e"].ap()[h],
                                          e_sb[:].bitcast(F32))
                        av_dbg = nrm.tile([D + 1, N], F32, tag="av_dbg",
                                          name="av_dbg")
                        nc.vector.tensor_copy(av_dbg[:], av[:])
                        nc.sync.dma_start(dbg["d_av"].ap()[h], av_dbg[:])
                    den0 = nrm.tile([1, N], F32, tag="den0", name="den0")
                    nc.scalar.copy(den0[:], av[D:D + 1, :])
                    rcp = nrm.tile([1, N], F32, tag="rcp", name="rcp")
                    nc.vector.reciprocal_approx_fast(rcp[:], den0[:])
                    rbc = nrm.tile([D, N], F32, tag="rbc", name="rbc")
                    nc.gpsimd.partition_broadcast(rbc[:], rcp[:])
                    nc.vector.tensor_mul(
                        s["ot"][(h % 2) * D:(h % 2) * D + D, h // 2, :],
                        av[0:D, :], rbc[:])

            def final(b):
                s = st[b]
                if debug_taps and b == 0:
                    nc.sync.dma_start(dbg["d_qt"].ap(),
                                      s["qt"][:].bitcast(F32))
                    nc.sync.dma_start(dbg["d_kt"].ap(),
                                      s["kt"][:].bitcast(F32))
                    nc.sync.dma_start(dbg["d_v"].ap(), s["v"][:].bitcast(F32))
                    nc.sync.dma_start(dbg["d_ot"].ap(),
                                      s["ot"][:].bitcast(F32))
                for it in range(NT):
                    ps = mmps.tile([P, N], F32, tag="proj", name="ps")
                    for et in range(ET):
                        nc.tensor.matmul(
                            ps[:],
                            s["ot"][:, et, it * P:(it + 1) * P],
                            w_sb["wo"][:, et, :],
                            start=(et == 0), stop=(et == ET - 1))
                    o_sb = opool.tile([P, N], F32, tag="o", name="o_sb")
                    nc.vector.tensor_add(o_sb[:], ps[:], bo_sb[:])
                    nc.sync.dma_start(out.ap()[b, it * P:(it + 1) * P, :],
                                      o_sb[:])

            # ---- emission schedule (PE program order) ----
            load(0, first=True)
            proj(0)
            st[0]["ot"] = otpool.tile([P, ET, N], F32R, tag="ot", name="ot0")
            e_prev = scores(0, 0)
            for hp in range(1, HP):
                e_cur = scores(0, hp)
                av_norm(0, hp - 1, e_prev)
                e_prev = e_cur
            load(1)
            proj(1)  # fills the PE while batch 0's last exp chain drains
            av_norm(0, HP - 1, e_prev)
            st[1]["ot"] = otpool.tile([P, ET, N], F32R, tag="ot", name="ot1")
            e_prev = scores(1, 0)
            final(0)  # after scores(1,0) so the PE rides over b0's norm tail
            for hp in range(1, HP):
                e_cur = scores(1, hp)
                av_norm(1, hp - 1, e_prev)
                e_prev = e_cur
            av_norm(1, HP - 1, e_prev)
            final(1)
    nc.compile()
    return nc


_NC = None


def _get_nc():
    global _NC
    if _NC is None:
        _NC = build_nc()
    return _NC


def _prep_in_maps(x, db, mask, wq, bq, wk, bk, wv, bv, wo, bo):
    f = np.float32
    x = np.asarray(x, f)
    db = np.asarray(db, f)
    mask = np.asarray(mask)
    xTa = np.ascontiguousarray(x.transpose(0, 2, 1))
    dbTa = np.ascontiguousarray(db.transpose(0, 2, 1))
    m01a = (mask != 0).astype(f)
    consts = dict(
        wqT=np.ascontiguousarray(np.asarray(wq, f).T),
        wkT=np.ascontiguousarray(np.asarray(wk, f).T),
        wvT=np.ascontiguousarray(np.asarray(wv, f).T),
        woT=np.ascontiguousarray(np.asarray(wo, f).T),
        bq=np.asarray(bq, f),
        bk=np.asarray(bk, f),
        bv=np.asarray(bv, f),
        bo=np.asarray(bo, f),
        ident=np.eye(P, dtype=f),
    )
    in_maps = []
    for c in range(NCORES):
        s = slice(c * BPC, (c + 1) * BPC)
        in_maps.append(dict(xT=xTa[s], dbT=dbTa[s], m01=m01a[s], **consts))
    return in_maps


def _install_ntff_hook():
    """The agent image's antenv lacks axon_hooks; provide a shim so
    run_bass_kernel_spmd(trace=True) can capture NTFF profiles."""
    import types

    if "antenv.axon_hooks" in sys.modules:
        return
    try:
        from trn_agent_boot.trn_boot import _ntff_profile_via_ctypes
        hook = _ntff_profile_via_ctypes("/opt/axon/libaxon_pjrt.so")
    except Exception:
        hook = None
    mod = types.ModuleType("antenv.axon_hooks")
    mod.get_axon_ntff_profile_hook = lambda: hook
    mod.set_axon_ntff_profile_hook = lambda h: None
    sys.modules["antenv.axon_hooks"] = mod


def run(trace=False, **inputs):
    if trace:
        _install_ntff_hook()
    nc = _get_nc()
    in_maps = _prep_in_maps(**inputs)
    res = run_bass_kernel_spmd(nc, in_maps, core_ids=list(range(NCORES)),
                               trace=trace)
    out = np.concatenate([res.results[c]["out"] for c in range(NCORES)],
                         axis=0)
    return out, res


def kernel(**inputs):
    out, _ = run(trace=False, **inputs)
    return out


# revision 28
# speedup vs baseline: 1.0498x; 1.0079x over previous
"""Trainium2 Bass kernel for DistanceSelfAttention.

Computation (per batch b):
    q/k/v = x @ w{q,k,v}.T + b{q,k,v}            -> [N, E], heads H=8, D=64
    sc    = clip(q k^T / sqrt(D) + db, -10, 10)
    sc    = where(mask[j], sc, -1e9)             (key-side mask)
    a     = softmax(sc, axis=-1)
    out   = (a v) @ wo.T + bo

Sharding: pure data-parallel over batch B=16 across 8 cores (2 per core),
weights replicated, no collectives.

Device-side design (per local batch):
    xT  [e, i]   - x transposed (host-prepped)
    QT/KT [e',i] - projections with output-dim on partitions; bias (+1/sqrt(D)
                   scale for Q) fused into the PSUM->SBUF evacuation
    V   [j, e_v] - token-major, head-split with a trailing ones column, rows
                   scaled by the 0/1 key mask: the AV matmul then yields both
                   the masked numerator and the masked softmax denominator
                   (last PSUM row) in one pass
    S.T [j, i]   - scores transposed; db.T injected into each head's PSUM via
                   an identity matmul, then the K=64 QK product accumulates
                   on top.  exp runs on ACT directly from PSUM; clip is
                   applied *after* exp as clamp(e, e^-10, e^+10) on GpSimd
                   (monotonicity makes them equivalent; masking lives in V)
    O.T [e, i]   - AV output, normalized by reciprocal denominators broadcast
                   across partitions
All matmuls run as float32r (TF32-like, 1 PE cycle/row vs 4 for fp32).
Emission is software-pipelined: head-pair p+1's scores are issued before
pair p's AV, and batch 1's projections are issued inside batch 0's
attention tail so the PE never drains.
"""

import sys

sys.path.insert(0, "/opt/trn_rl_repo")

import numpy as np

import concourse.bass as bass  # noqa: F401
import concourse.tile as tile
from concourse import bacc, mybir
from concourse.bass_utils import run_bass_kernel_spmd

B, N, E, H = 16, 512, 512, 8
D = E // H
P = 128
NCORES = 8
BPC = B // NCORES  # batches per core
NT = N // P        # token tiles
ET = E // P        # embedding tiles
HP = H // 2        # head pairs
F32 = mybir.dt.float32
F32R = mybir.dt.float32r
BF16 = mybir.dt.bfloat16
AX = mybir.AluOpType
AF = mybir.ActivationFunctionType
EXP10 = float(np.exp(10.0))
EXPM10 = float(np.exp(-10.0))


def build_nc(debug_taps=False):
    nc = bacc.Bacc("TRN2", target_bir_lowering=False, debug=False,
                   num_devices=NCORES)

    xT = nc.dram_tensor("xT", [BPC, E, N], F32R, kind="ExternalInput")
    dbT = nc.dram_tensor("dbT", [BPC, N, N], F32R, kind="ExternalInput")
    m01 = nc.dram_tensor("m01", [BPC, N], F32, kind="ExternalInput")
    wqT = nc.dram_tensor("wqT", [E, E], F32R, kind="ExternalInput")
    wkT = nc.dram_tensor("wkT", [E, E], F32R, kind="ExternalInput")
    wvT = nc.dram_tensor("wvT", [E, E], F32R, kind="ExternalInput")
    woT = nc.dram_tensor("woT", [E, E], F32R, kind="ExternalInput")
    bq = nc.dram_tensor("bq", [E], F32, kind="ExternalInput")
    bk = nc.dram_tensor("bk", [E], F32, kind="ExternalInput")
    bv = nc.dram_tensor("bv", [E], F32, kind="ExternalInput")
    bo = nc.dram_tensor("bo", [E], F32, kind="ExternalInput")
    ident = nc.dram_tensor("ident", [P, P], F32R, kind="ExternalInput")
    out = nc.dram_tensor("out", [BPC, N, E], F32, kind="ExternalOutput")

    with tile.TileContext(nc) as tc:
        with (
            tc.tile_pool(name="wpool", bufs=1) as wpool,
            tc.tile_pool(name="cpool", bufs=1) as cpool,
            tc.tile_pool(name="xpool", bufs=2) as xpool,
            tc.tile_pool(name="dbpool", bufs=2) as dbpool,
            tc.tile_pool(name="qkpool", bufs=2) as qkpool,
            tc.tile_pool(name="vpool", bufs=2) as vpool,
            tc.tile_pool(name="epool", bufs=2) as epool,
            tc.tile_pool(name="otpool", bufs=2) as otpool,
            tc.tile_pool(name="nrm", bufs=3) as nrm,
            tc.tile_pool(name="opool", bufs=3) as opool,
            tc.tile_pool(name="scps", bufs=2, space="PSUM") as scps,
            tc.tile_pool(name="avps", bufs=2, space="PSUM") as avps,
            tc.tile_pool(name="mmps", bufs=2, space="PSUM") as mmps,
        ):
            # ---- resident weights / constants ----
            # Weights stream on the ACT HWDGE ring (idle during the head);
            # xT/dbT stream on the SP ring so the first projection matmul
            # is gated only by wq chunk 0 + xT chunk 0.
            w_sb = {}
            w_src = {}
            for name, t in (("wq", wqT), ("wk", wkT), ("wv", wvT),
                            ("wo", woT)):
                w_sb[name] = wpool.tile([P, ET, E], F32R, tag=f"w_{name}",
                                        name=name)
                w_src[name] = t.ap().rearrange("(kt p) o -> p kt o", p=P)

            def load_w(name, engs=(None,)):
                for kt in range(ET):
                    eng = engs[kt % len(engs)]
                    eng.dma_start(w_sb[name][:, kt, :],
                                  w_src[name][:, kt, :])

            for _wn in ("wq", "wk", "wv", "wo"):
                load_w(_wn, (nc.scalar,))
            bq_sb = cpool.tile([P, ET], F32, tag="bq")
            nc.gpsimd.dma_start(bq_sb[:],
                                bq.ap().rearrange("(t p) -> p t", p=P))
            bk_sb = cpool.tile([P, ET], F32, tag="bk")
            nc.gpsimd.dma_start(bk_sb[:],
                                bk.ap().rearrange("(t p) -> p t", p=P))
            bv_sb = cpool.tile([P, E], F32, tag="bv")
            nc.gpsimd.dma_start(bv_sb[:],
                                bv.ap()[None, :].broadcast_to([P, E]))
            bo_sb = cpool.tile([P, E], F32, tag="bo")
            nc.gpsimd.dma_start(bo_sb[:],
                                bo.ap()[None, :].broadcast_to([P, E]))
            id_sb = cpool.tile([P, P], F32R, tag="ident")
            nc.gpsimd.dma_start(id_sb[:], ident.ap())

            dbg = {}
            if debug_taps:
                for nm, shp in (("d_qt", [P, ET, N]), ("d_kt", [P, ET, N]),
                                ("d_v", [P, NT, H, D + 1]),
                                ("d_e", [H, P, NT, N]),
                                ("d_av", [H, D + 1, N]),
                                ("d_ot", [P, ET, N])):
                    dbg[nm] = nc.dram_tensor(nm, shp, F32,
                                             kind="ExternalOutput")

            st = [dict() for _ in range(BPC)]  # per-batch live tiles

            def load(b, first=False):
                xT_sb = xpool.tile([P, ET, N], F32R, tag="xT", name="xT_sb")
                xr = xT.ap()[b].rearrange("(kt p) i -> p kt i", p=P)
                for kt in range(ET):
                    nc.sync.dma_start(xT_sb[:, kt, :], xr[:, kt, :])
                dbT_sb = dbpool.tile([P, NT, N], F32R, tag="dbT",
                                     name="dbT_sb")
                dr = dbT.ap()[b].rearrange("(jt p) i -> p jt i", p=P)
                for jt in range(NT):
                    nc.sync.dma_start(dbT_sb[:, jt, :], dr[:, jt, :])
                m01_sb = cpool.tile([P, NT], F32, tag=f"m01{b}",
                                    name="m01_sb")
                nc.gpsimd.dma_start(m01_sb[:], m01.ap()[b].rearrange(
                    "(t p) -> p t", p=P))
                st[b].update(xT=xT_sb, dbT=dbT_sb, m01=m01_sb)

            def proj(b):
                s = st[b]
                xT_sb = s["xT"]
                qt_sb = qkpool.tile([P, ET, N], F32R, tag="qt", name="qt_sb")
                kt_sb = qkpool.tile([P, ET, N], F32R, tag="kt", name="kt_sb")
                for wname, dst, bias, scl in (
                        ("wq", qt_sb, bq_sb, float(1 / np.sqrt(D))),
                        ("wk", kt_sb, bk_sb, None)):
                    for et in range(ET):
                        ps = mmps.tile([P, N], F32, tag="proj", name="ps")
                        for ke in range(ET):
                            nc.tensor.matmul(
                                ps[:],
                                w_sb[wname][:, ke, et * P:(et + 1) * P],
                                xT_sb[:, ke, :],
                                start=(ke == 0), stop=(ke == ET - 1))
                        if scl is not None:
                            nc.vector.tensor_scalar(
                                dst[:, et, :], ps[:], bias[:, et:et + 1],
                                scl, AX.add, AX.mult)
                        else:
                            nc.vector.tensor_scalar(
                                dst[:, et, :], ps[:], bias[:, et:et + 1],
                                None, AX.add)
                v_sb = vpool.tile([P, NT, H, D + 1], BF16, tag="v",
                                  name="v_sb")
                for jt in range(NT):
                    ps = mmps.tile([P, N], F32, tag="proj", name="ps")
                    for ke in range(ET):
                        nc.tensor.matmul(
                            ps[:],
                            xT_sb[:, ke, jt * P:(jt + 1) * P],
                            w_sb["wv"][:, ke, :],
                            start=(ke == 0), stop=(ke == ET - 1))
                    nc.vector.tensor_add(
                        v_sb[:, jt, :, 0:D],
                        ps[:].rearrange("p (h d) -> p h d", h=H),
                        bv_sb[:].rearrange("p (h d) -> p h d", h=H))
                    nc.vector.memset(v_sb[:, jt, :, D:D + 1], 1.0)
                    # key mask: zero masked token rows (incl. ones column)
                    nc.vector.tensor_scalar(
                        v_sb[:, jt, :, :], v_sb[:, jt, :, :],
                        s["m01"][:, jt:jt + 1], None, AX.mult)
                s.update(qt=qt_sb, kt=kt_sb, v=v_sb)

            def scores(b, hp):
                """Head pair (2hp, 2hp+1): db-injected, exp'd score tiles."""
                s = st[b]
                e_ab = (epool.tile([P, NT, N], BF16, tag="eA", name="eA"),
                        epool.tile([P, NT, N], BF16, tag="eB", name="eB"))
                for jt in range(NT):
                    sc_ab = (scps.tile([P, N], F32, tag="scA", name="scA"),
                             scps.tile([P, N], F32, tag="scB", name="scB"))
                    for sc in sc_ab:
                        nc.tensor.matmul(sc[:], id_sb[:], s["dbT"][:, jt, :],
                                         start=True, stop=False)
                    for half, sc in enumerate(sc_ab):
                        of = half * D
                        nc.tensor.matmul(
                            sc[:],
                            s["kt"][of:of + D, hp, jt * P:(jt + 1) * P],
                            s["qt"][of:of + D, hp, :],
                            start=False, stop=True, tile_position=(of, 0))
                    for half, sc in enumerate(sc_ab):
                        nc.scalar.activation(e_ab[half][:, jt, :], sc[:],
                                             AF.Exp)
                        nc.vector.tensor_scalar(
                            e_ab[half][:, jt, :], e_ab[half][:, jt, :],
                            EXP10, EXPM10, AX.min, AX.max)
                return e_ab

            def av_norm(b, hp, e_ab):
                s = st[b]
                for half, e_sb in enumerate(e_ab):
                    h = 2 * hp + half
                    av = avps.tile([D + 1, N], F32, tag="av", name="av")
                    for jt in range(NT):
                        nc.tensor.matmul(av[:], s["v"][:, jt, h, :],
                                         e_sb[:, jt, :],
                                         start=(jt == 0), stop=(jt == NT - 1))
                    if debug_taps and b == 0:
                        nc.sync.dma_start(dbg["d_e"].ap()[h],
                                          e_sb[:].bitcast(F32))
                        av_dbg = nrm.tile([D + 1, N], F32, tag="av_dbg",
                                          name="av_dbg")
                        nc.vector.tensor_copy(av_dbg[:], av[:])
                        nc.sync.dma_start(dbg["d_av"].ap()[h], av_dbg[:])
                    den0 = nrm.tile([1, N], F32, tag="den0", name="den0")
                    nc.scalar.copy(den0[:], av[D:D + 1, :])
                    rcp = nrm.tile([1, N], F32, tag="rcp", name="rcp")
                    nc.vector.reciprocal_approx_fast(rcp[:], den0[:])
                    rbc = nrm.tile([D, N], F32, tag="rbc", name="rbc")
                    nc.gpsimd.partition_broadcast(rbc[:], rcp[:])
                    nc.vector.tensor_mul(
                        s["ot"][(h % 2) * D:(h % 2) * D + D, h // 2, :],
                        av[0:D, :], rbc[:])

            def final(b):
                s = st[b]
                if debug_taps and b == 0:
                    nc.sync.dma_start(dbg["d_qt"].ap(),
                                      s["qt"][:].bitcast(F32))
                    nc.sync.dma_start(dbg["d_kt"].ap(),
                                      s["kt"][:].bitcast(F32))
                    nc.sync.dma_start(dbg["d_v"].ap(), s["v"][:].bitcast(F32))
                    nc.sync.dma_start(dbg["d_ot"].ap(),
                                      s["ot"][:].bitcast(F32))
                for it in range(NT):
                    ps = mmps.tile([P, N], F32, tag="proj", name="ps")
                    for et in range(ET):
                        nc.tensor.matmul(
                            ps[:],
                            s["ot"][:, et, it * P:(it + 1) * P],
                            w_sb["wo"][:, et, :],
                            start=(et == 0), stop=(et == ET - 1))
                    o_sb = opool.tile([P, N], F32, tag="o", name="o_sb")
                    nc.vector.tensor_add(o_sb[:], ps[:], bo_sb[:])
                    nc.sync.dma_start(out.ap()[b, it * P:(it + 1) * P, :],
                                      o_sb[:])

            # ---- emission schedule (PE program order) ----
            load(0, first=True)
            proj(0)
            st[0]["ot"] = otpool.tile([P, ET, N], F32R, tag="ot", name="ot0")
            e_prev = scores(0, 0)
            for hp in range(1, HP):
                e_cur = scores(0, hp)
                av_norm(0, hp - 1, e_prev)
                e_prev = e_cur
            load(1)
            proj(1)  # fills the PE while batch 0's last exp chain drains
            av_norm(0, HP - 1, e_prev)
            st[1]["ot"] = otpool.tile([P, ET, N], F32R, tag="ot", name="ot1")
            e_prev = scores(1, 0)
            final(0)  # after scores(1,0) so the PE rides over b0's norm tail
            for hp in range(1, HP):
                e_cur = scores(1, hp)
                av_norm(1, hp - 1, e_prev)
                e_prev = e_cur
            av_norm(1, HP - 1, e_prev)
            final(1)
    nc.compile()
    return nc


_NC = None


def _get_nc():
    global _NC
    if _NC is None:
        _NC = build_nc()
    return _NC


def _prep_in_maps(x, db, mask, wq, bq, wk, bk, wv, bv, wo, bo):
    f = np.float32
    x = np.asarray(x, f)
    db = np.asarray(db, f)
    mask = np.asarray(mask)
    xTa = np.ascontiguousarray(x.transpose(0, 2, 1))
    dbTa = np.ascontiguousarray(db.transpose(0, 2, 1))
    m01a = (mask != 0).astype(f)
    consts = dict(
        wqT=np.ascontiguousarray(np.asarray(wq, f).T),
        wkT=np.ascontiguousarray(np.asarray(wk, f).T),
        wvT=np.ascontiguousarray(np.asarray(wv, f).T),
        woT=np.ascontiguousarray(np.asarray(wo, f).T),
        bq=np.asarray(bq, f),
        bk=np.asarray(bk, f),
        bv=np.asarray(bv, f),
        bo=np.asarray(bo, f),
        ident=np.eye(P, dtype=f),
    )
    in_maps = []
    for c in range(NCORES):
        s = slice(c * BPC, (c + 1) * BPC)
        in_maps.append(dict(xT=xTa[s], dbT=dbTa[s], m01=m01a[s], **consts))
    return in_maps


def _install_ntff_hook():
    """The agent image's antenv lacks axon_hooks; provide a shim so
    run_bass_kernel_spmd(trace=True) can capture NTFF profiles."""
    import types

    if "antenv.axon_hooks" in sys.modules:
        return
    try:
        from trn_agent_boot.trn_boot import _ntff_profile_via_ctypes
        hook = _ntff_profile_via_ctypes("/opt/axon/libaxon_pjrt.so")
    except Exception:
        hook = None
    mod = types.ModuleType("antenv.axon_hooks")
    mod.get_axon_ntff_profile_hook = lambda: hook
    mod.set_axon_ntff_profile_hook = lambda h: None
    sys.modules["antenv.axon_hooks"] = mod


def run(trace=False, **inputs):
    if trace:
        _install_ntff_hook()
    nc = _get_nc()
    in_maps = _prep_in_maps(**inputs)
    res = run_bass_kernel_spmd(nc, in_maps, core_ids=list(range(NCORES)),
                               trace=trace)
    out = np.concatenate([res.results[c]["out"] for c in range(NCORES)],
                         axis=0)
    return out, res


def kernel(**inputs):
    out, _ = run(trace=False, **inputs)
    return out
